# revision 18
# baseline (speedup 1.0000x reference)
"""BoxBlur 13x13 depthwise conv (reflect pad) on 8 trn2 NeuronCores.

Input (8, 64, 512, 512) f32 + kernel (1, 13, 13) f32 -> output (8, 64, 512, 512).

Sharding: batch dim across 8 cores (one sample = 64 channel-images per core).

Algorithm (per 512x512 image): box blur is separable. Both 1D 13-tap passes
(reflect padding folded into an integer band matrix M[h, h'] built on host)
run on the tensor engine as normal-mode matmuls with the image block as the
STATIONARY operand and the band matrix as the MOVING operand, which fuses a
transpose into each pass:

    pass1:  Y1t[w, h'] = sum_h X[h, w] * M[h, h']      (vconv, output transposed)
    pass2:  out[h', w'] = sum_w Y1t[w, h'] * M[w, w']  (hconv, transpose undone)

Each pass is 4 contraction blocks x 4 stationary 128-slices = 16 matmuls per
image, PSUM-accumulated over the contraction blocks using partial-range
windows (the band is zero outside a ~140-wide window per block). The 1/169
scale is folded into the final PSUM->SBUF evacuation.

The whole pipeline runs in fp16 (rel_max ~5e-4 vs the f32 reference):
the host rounds the input to fp16 and repacks it into a DMA-friendly
layout [NBLK, 128, C*W] so every load/store moves G=8 images' worth of a
row-block in one 1 MiB DMA with 8 KB contiguous per partition line; fp16
also runs every matmul at full PE speed (f32 matmuls are 4x slower) and
halves HBM traffic, which is the roofline for this kernel.
"""
import numpy as np

B, C, H, W = 8, 64, 512, 512
KY = KX = 13
HALF = 6
N_CORES = 8
P = 128
NBLK = H // P  # 4
G = 8          # images per DMA group

# per contraction block k: window [start, width) of nonzero band columns
_WINDOWS = [
    (max(0, P * k - HALF),
     min(H, P * k + P - 1 + HALF + 1) - max(0, P * k - HALF))
    for k in range(NBLK)
]


def _band_matrix() -> np.ndarray:
    """M[h, h'] = number of taps of output h' that hit input row h
    (13-tap, reflect padding, pad = 6 both sides)."""
    m = np.zeros((H, H), dtype=np.float32)
    for hp in range(H):
        for d in range(-HALF, HALF + 1):
            h = hp + d
            if h < 0:
                h = -h
            if h > H - 1:
                h = 2 * (H - 1) - h
            m[h, hp] += 1.0
    return m


def _build_nc(scale: float, n_images: int):
    import concourse.bacc as bacc
    import concourse.mybir as mybir
    from concourse.tile import TileContext

    f16 = mybir.dt.float16
    nc = bacc.Bacc(trn_type="TRN2")

    CW = n_images * W
    GW = G * W
    x = nc.dram_tensor("x", [NBLK, P, CW], f16, kind="ExternalInput")
    band = [
        nc.dram_tensor(f"band{k}", [P, _WINDOWS[k][1]], f16, kind="ExternalInput")
        for k in range(NBLK)
    ]
    y = nc.dram_tensor("y", [NBLK, P, CW], f16, kind="ExternalOutput")

    with TileContext(nc) as tc:
        with (
            tc.tile_pool(name="const", bufs=1) as const_pool,
            tc.tile_pool(name="xin", bufs=3) as x_pool,
            tc.tile_pool(name="mid", bufs=6) as mid_pool,
            tc.tile_pool(name="oout", bufs=2) as out_pool,
            tc.tile_pool(name="ps1", bufs=2, space="PSUM") as ps1_pool,
            tc.tile_pool(name="ps2", bufs=2, space="PSUM") as ps2_pool,
        ):
            band_t = []
            for k in range(NBLK):
                bt = const_pool.tile([P, _WINDOWS[k][1]], f16, tag=f"band{k}")
                nc.sync.dma_start(bt[:], band[k][:])
                band_t.append(bt)

            n_groups = n_images // G
            HW_ = GW // 2  # half-group columns per DMA

            def load_group(g):
                # all h0 halves first: the group's first images only need
                # those, so compute can start after half the load bytes
                xts = [x_pool.tile([P, GW], f16, name=f"xt{k}", tag=f"x{k}")
                       for k in range(NBLK)]
                for h in range(2):
                    for k in range(NBLK):
                        nc.sync.dma_start(
                            xts[k][:, h * HW_:(h + 1) * HW_],
                            x[k, :, g * GW + h * HW_:g * GW + (h + 1) * HW_])
                return xts

            QW_ = GW // 4  # quarter-group columns per store DMA

            def store_group(g, ots, q):
                # stores issue from the sync engine too: issuing them from
                # ACT blocks the in-order ACT sequencer (and with it all
                # later PSUM evacuations) whenever a store's wait condition
                # is still pending.  Quarter-group granularity keeps the
                # drain tracking the evacuation engines instead of waiting
                # on half-group batches (shrinks the tail).
                for i in range(NBLK):
                    ii, i2 = divmod(i, 2)
                    nc.sync.dma_start(
                        y[i, :, g * GW + q * QW_:g * GW + (q + 1) * QW_],
                        ots[ii][:, i2, q * QW_:(q + 1) * QW_])

            def do_pass1(xts, ci):
                # pass 1: Y1t_j[w, h'] = sum_h X[h, 128j + w] M[h, h']
                # Two j-blocks share one 2-bank PSUM tile so each PSUM
                # evacuation is a single 1024-wide op (amortizes the
                # fixed PSUM access latency on DVE/ACT).
                co = ci * W
                y1 = []
                for jj in range(2):
                    ps = ps1_pool.tile([P, 2, H], mybir.dt.float32)
                    for j2 in range(2):
                        j = 2 * jj + j2
                        for k in range(NBLK):
                            w0, wid = _WINDOWS[k]
                            nc.tensor.matmul(
                                ps[:, j2, w0:w0 + wid],
                                xts[k][:, co + P * j:co + P * (j + 1)],
                                band_t[k][:],
                                start=(k == 0), stop=(k == NBLK - 1),
                            )
                    yt = mid_pool.tile([P, 2, H], f16)
                    if jj == 0:
                        nc.vector.tensor_copy(yt[:], ps[:])
                    else:
                        nc.scalar.copy(yt[:], ps[:])
                    y1.append(yt)
                return y1

            def do_pass2(y1, ots, ci):
                # pass 2: out_i[h', w'] = sum_w Y1t[w, 128i + h'] M[w, w']
                co = ci * W
                for ii in range(2):
                    ps = ps2_pool.tile([P, 2, W], mybir.dt.float32, tag="ps2")
                    for i2 in range(2):
                        i = 2 * ii + i2
                        for j in range(NBLK):
                            w0, wid = _WINDOWS[j]
                            jj, j2 = divmod(j, 2)
                            nc.tensor.matmul(
                                ps[:, i2, w0:w0 + wid],
                                y1[jj][:, j2, P * i:P * (i + 1)],
                                band_t[j][:],
                                start=(j == 0), stop=(j == NBLK - 1),
                            )
                    if ii == 0:
                        nc.scalar.mul(ots[ii][:, :, co:co + W], ps[:], scale)
                    else:
                        nc.vector.tensor_scalar_mul(
                            ots[ii][:, :, co:co + W], ps[:], scale)

            # Software pipeline across images: pass1 of image n+1 is
            # emitted before pass2 of image n, so the PE never waits on
            # the DVE/ACT evacuation of the intermediate.  Input loads
            # ride the sync HWDGE ring, output stores the scalar ring;
            # each group's loads prefetch two groups ahead, and stores
            # drain per half-group as soon as its evacuations land.
            xts_by_g = {0: load_group(0)}
            if n_groups > 1:
                xts_by_g[1] = load_group(1)
            ots_by_g = {}
            prev = None
            for n in range(n_images):
                g, ci = divmod(n, G)
                if ci == 0:
                    if g + 2 < n_groups:
                        xts_by_g[g + 2] = load_group(g + 2)
                    ots_by_g[g] = [
                        out_pool.tile([P, 2, GW], f16, name=f"ot{ii}",
                                      tag=f"o{ii}")
                        for ii in range(2)
                    ]
                y1 = do_pass1(xts_by_g[g], ci)
                if prev is not None:
                    pg, pci, py1 = prev
                    do_pass2(py1, ots_by_g[pg], pci)
                    if pci % 2 == 1:
                        store_group(pg, ots_by_g[pg], pci // 2)
                    if pci == G - 1:
                        xts_by_g.pop(pg)
                prev = (g, ci, y1)
            pg, pci, py1 = prev
            do_pass2(py1, ots_by_g[pg], pci)
            store_group(pg, ots_by_g[pg], pci // 2)

    nc.compile()
    return nc


def _pack(xb: np.ndarray) -> np.ndarray:
    """(C, H, W) f32 -> [NBLK, P, C*W] f16, row-block-major."""
    a = xb.astype(np.float16).reshape(C, NBLK, P, W).transpose(1, 2, 0, 3)
    return np.ascontiguousarray(a).reshape(NBLK, P, C * W)


def _unpack(yb: np.ndarray) -> np.ndarray:
    """[NBLK, P, C*W] f16 -> (C, H, W) f32."""
    a = np.asarray(yb).reshape(NBLK, P, C, W).transpose(2, 0, 1, 3)
    return np.ascontiguousarray(a).reshape(C, H, W).astype(np.float32)


def _run(inputs: dict, mode: str = "f16", trace: bool = False):
    from concourse.bass_utils import run_bass_kernel_spmd

    x = np.asarray(inputs["input"], dtype=np.float32)
    ker = np.asarray(inputs["kernel"], dtype=np.float32)
    scale = float(ker[0, 0, 0])

    m = _band_matrix()
    bands = [
        np.ascontiguousarray(
            m[P * k:P * (k + 1),
              _WINDOWS[k][0]:_WINDOWS[k][0] + _WINDOWS[k][1]]
        ).astype(np.float16)
        for k in range(NBLK)
    ]

    nc = _build_nc(scale, C)
    in_maps = []
    for b in range(B):
        im = {"x": _pack(x[b])}
        for k in range(NBLK):
            im[f"band{k}"] = bands[k]
        in_maps.append(im)

    res = run_bass_kernel_spmd(nc, in_maps, core_ids=list(range(N_CORES)),
                               trace=trace)
    out = np.stack([_unpack(res.results[b]["y"]) for b in range(B)], axis=0)
    return out, res


def kernel(**inputs) -> np.ndarray:
    out, _ = _run(inputs)
    return out


# revision 22
# speedup vs baseline: 1.0500x; 1.0500x over previous
"""BoxBlur 13x13 depthwise conv (reflect pad) on 8 trn2 NeuronCores.

Input (8, 64, 512, 512) f32 + kernel (1, 13, 13) f32 -> output (8, 64, 512, 512).

Sharding: batch dim across 8 cores (one sample = 64 channel-images per core).

Algorithm (per 512x512 image): box blur is separable. Both 1D 13-tap passes
(reflect padding folded into an integer band matrix M[h, h'] built on host)
run on the tensor engine as normal-mode matmuls with the image block as the
STATIONARY operand and the band matrix as the MOVING operand, which fuses a
transpose into each pass:

    pass1:  Y1t[w, h'] = sum_h X[h, w] * M[h, h']      (vconv, output transposed)
    pass2:  out[h', w'] = sum_w Y1t[w, h'] * M[w, w']  (hconv, transpose undone)

Each pass is 4 contraction blocks x 4 stationary 128-slices = 16 matmuls per
image, PSUM-accumulated over the contraction blocks using partial-range
windows (the band is zero outside a ~140-wide window per block). The 1/169
scale is folded into the final PSUM->SBUF evacuation.

The whole pipeline runs in fp16 (rel_max ~5e-4 vs the f32 reference):
the host rounds the input to fp16 and repacks it into a DMA-friendly
layout [NBLK, 128, C*W] so every load/store moves G=8 images' worth of a
row-block in one 1 MiB DMA with 8 KB contiguous per partition line; fp16
also runs every matmul at full PE speed (f32 matmuls are 4x slower) and
halves HBM traffic, which is the roofline for this kernel.
"""
import numpy as np

B, C, H, W = 8, 64, 512, 512
KY = KX = 13
HALF = 6
N_CORES = 8
P = 128
NBLK = H // P  # 4
G = 8          # images per DMA group

# per contraction block k: window [start, width) of nonzero band columns
_WINDOWS = [
    (max(0, P * k - HALF),
     min(H, P * k + P - 1 + HALF + 1) - max(0, P * k - HALF))
    for k in range(NBLK)
]


def _band_matrix() -> np.ndarray:
    """M[h, h'] = number of taps of output h' that hit input row h
    (13-tap, reflect padding, pad = 6 both sides)."""
    m = np.zeros((H, H), dtype=np.float32)
    for hp in range(H):
        for d in range(-HALF, HALF + 1):
            h = hp + d
            if h < 0:
                h = -h
            if h > H - 1:
                h = 2 * (H - 1) - h
            m[h, hp] += 1.0
    return m


def _build_nc(scale: float, n_images: int):
    import concourse.bacc as bacc
    import concourse.mybir as mybir
    from concourse.tile import TileContext

    f16 = mybir.dt.float16
    nc = bacc.Bacc(trn_type="TRN2")

    CW = n_images * W
    GW = G * W
    x = nc.dram_tensor("x", [NBLK, P, CW], f16, kind="ExternalInput")
    band = [
        nc.dram_tensor(f"band{k}", [P, _WINDOWS[k][1]], f16, kind="ExternalInput")
        for k in range(NBLK)
    ]
    y = nc.dram_tensor("y", [NBLK, P, CW], f16, kind="ExternalOutput")

    with TileContext(nc) as tc:
        with (
            tc.tile_pool(name="const", bufs=1) as const_pool,
            tc.tile_pool(name="xin", bufs=2) as x_pool,
            tc.tile_pool(name="mid", bufs=6) as mid_pool,
            tc.tile_pool(name="oout", bufs=2) as out_pool,
            tc.tile_pool(name="ps1", bufs=2, space="PSUM") as ps1_pool,
            tc.tile_pool(name="ps2", bufs=2, space="PSUM") as ps2_pool,
        ):
            band_t = []
            for k in range(NBLK):
                bt = const_pool.tile([P, _WINDOWS[k][1]], f16, tag=f"band{k}")
                nc.sync.dma_start(bt[:], band[k][:])
                band_t.append(bt)

            n_groups = n_images // G
            HW_ = GW // 2  # half-group columns per DMA

            def load_group(g):
                # all h0 halves first: the group's first images only need
                # those, so compute can start after half the load bytes
                xts = [x_pool.tile([P, GW], f16, name=f"xt{k}", tag=f"x{k}")
                       for k in range(NBLK)]
                for h in range(2):
                    for k in range(NBLK):
                        nc.sync.dma_start(
                            xts[k][:, h * HW_:(h + 1) * HW_],
                            x[k, :, g * GW + h * HW_:g * GW + (h + 1) * HW_])
                return xts

            def store_group(g, ots, h, splits=1):
                # stores issue from the sync engine too: issuing them from
                # ACT blocks the in-order ACT sequencer (and with it all
                # later PSUM evacuations) whenever a store's wait condition
                # is still pending
                sw = HW_ // splits
                for sp in range(splits):
                    for i in range(NBLK):
                        ii, i2 = divmod(i, 2)
                        c0 = h * HW_ + sp * sw
                        nc.sync.dma_start(
                            y[i, :, g * GW + c0:g * GW + c0 + sw],
                            ots[ii][:, i2, c0:c0 + sw])

            def do_pass1(xts, ci):
                # pass 1: Y1t_j[w, h'] = sum_h X[h, 128j + w] M[h, h']
                # Two j-blocks share one 2-bank PSUM tile so each PSUM
                # evacuation is a single 1024-wide op (amortizes the
                # fixed PSUM access latency on DVE/ACT).
                co = ci * W
                y1 = []
                for jj in range(2):
                    ps = ps1_pool.tile([P, 2, H], mybir.dt.float32)
                    for j2 in range(2):
                        j = 2 * jj + j2
                        for k in range(NBLK):
                            w0, wid = _WINDOWS[k]
                            nc.tensor.matmul(
                                ps[:, j2, w0:w0 + wid],
                                xts[k][:, co + P * j:co + P * (j + 1)],
                                band_t[k][:],
                                start=(k == 0), stop=(k == NBLK - 1),
                            )
                    yt = mid_pool.tile([P, 2, H], f16)
                    if jj == 0:
                        nc.vector.tensor_copy(yt[:], ps[:])
                    else:
                        nc.scalar.copy(yt[:], ps[:])
                    y1.append(yt)
                return y1

            def do_pass2(y1, ots, ci):
                # pass 2: out_i[h', w'] = sum_w Y1t[w, 128i + h'] M[w, w']
                co = ci * W
                for ii in range(2):
                    ps = ps2_pool.tile([P, 2, W], mybir.dt.float32, tag="ps2")
                    for i2 in range(2):
                        i = 2 * ii + i2
                        for j in range(NBLK):
                            w0, wid = _WINDOWS[j]
                            jj, j2 = divmod(j, 2)
                            nc.tensor.matmul(
                                ps[:, i2, w0:w0 + wid],
                                y1[jj][:, j2, P * i:P * (i + 1)],
                                band_t[j][:],
                                start=(j == 0), stop=(j == NBLK - 1),
                            )
                    if ii == 0:
                        nc.scalar.mul(ots[ii][:, :, co:co + W], ps[:], scale)
                    else:
                        nc.vector.tensor_scalar_mul(
                            ots[ii][:, :, co:co + W], ps[:], scale)

            # Software pipeline across images: pass1 of image n+1 is
            # emitted before pass2 of image n, so the PE never waits on
            # the DVE/ACT evacuation of the intermediate.  Loads prefetch
            # one group ahead — spread across the whole run they share
            # HBM with the store stream instead of front-loading it and
            # leaving a long store-only (production-limited) tail.
            xts_by_g = {0: load_group(0)}
            ots_by_g = {}
            prev = None
            for n in range(n_images):
                g, ci = divmod(n, G)
                if ci == 0:
                    if g + 1 < n_groups:
                        xts_by_g[g + 1] = load_group(g + 1)
                    ots_by_g[g] = [
                        out_pool.tile([P, 2, GW], f16, name=f"ot{ii}",
                                      tag=f"o{ii}")
                        for ii in range(2)
                    ]
                y1 = do_pass1(xts_by_g[g], ci)
                if prev is not None:
                    pg, pci, py1 = prev
                    do_pass2(py1, ots_by_g[pg], pci)
                    if pci == G // 2 - 1:
                        store_group(pg, ots_by_g[pg], 0)
                    elif pci == G - 1:
                        store_group(pg, ots_by_g[pg], 1)
                        xts_by_g.pop(pg)
                prev = (g, ci, y1)
            pg, pci, py1 = prev
            do_pass2(py1, ots_by_g[pg], pci)
            store_group(pg, ots_by_g[pg], 1, splits=2)

    nc.compile()
    return nc


def _pack(xb: np.ndarray) -> np.ndarray:
    """(C, H, W) f32 -> [NBLK, P, C*W] f16, row-block-major."""
    a = xb.astype(np.float16).reshape(C, NBLK, P, W).transpose(1, 2, 0, 3)
    return np.ascontiguousarray(a).reshape(NBLK, P, C * W)


def _unpack(yb: np.ndarray) -> np.ndarray:
    """[NBLK, P, C*W] f16 -> (C, H, W) f32."""
    a = np.asarray(yb).reshape(NBLK, P, C, W).transpose(2, 0, 1, 3)
    return np.ascontiguousarray(a).reshape(C, H, W).astype(np.float32)


def _run(inputs: dict, mode: str = "f16", trace: bool = False):
    from concourse.bass_utils import run_bass_kernel_spmd

    x = np.asarray(inputs["input"], dtype=np.float32)
    ker = np.asarray(inputs["kernel"], dtype=np.float32)
    scale = float(ker[0, 0, 0])

    m = _band_matrix()
    bands = [
        np.ascontiguousarray(
            m[P * k:P * (k + 1),
              _WINDOWS[k][0]:_WINDOWS[k][0] + _WINDOWS[k][1]]
        ).astype(np.float16)
        for k in range(NBLK)
    ]

    nc = _build_nc(scale, C)
    in_maps = []
    for b in range(B):
        im = {"x": _pack(x[b])}
        for k in range(NBLK):
            im[f"band{k}"] = bands[k]
        in_maps.append(im)

    res = run_bass_kernel_spmd(nc, in_maps, core_ids=list(range(N_CORES)),
                               trace=trace)
    out = np.stack([_unpack(res.results[b]["y"]) for b in range(B)], axis=0)
    return out, res


def kernel(**inputs) -> np.ndarray:
    out, _ = _run(inputs)
    return out
